# revision 1
# baseline (speedup 1.0000x reference)
import numpy as np

N = 100000
E = 3200000
G = 256
HID = 128
NH = 4
HC = 32
L = 4
EPS = 1e-5


def _bn(x, g, b):
    m = x.mean(0)
    v = x.var(0)
    return (x - m) / np.sqrt(v + EPS) * g + b


def _silu(x):
    return x / (1.0 + np.exp(-x))


def kernel(x, pos, edge_index, batch, W_in, b_in, in_g, in_b,
           Wg, a_src, a_dst, bg, ln_g, ln_b,
           W_att, b_att, W1, b1, W2, b2, W3, b3):
    x = np.asarray(x, np.float32); pos = np.asarray(pos, np.float32)
    edge_index = np.asarray(edge_index)
    batch = np.asarray(batch).astype(np.int64)
    W_in = np.asarray(W_in, np.float32); b_in = np.asarray(b_in, np.float32)
    in_g = np.asarray(in_g, np.float32); in_b = np.asarray(in_b, np.float32)
    Wg = np.asarray(Wg, np.float32)
    a_src = np.asarray(a_src, np.float32); a_dst = np.asarray(a_dst, np.float32)
    bg = np.asarray(bg, np.float32)
    ln_g = np.asarray(ln_g, np.float32); ln_b = np.asarray(ln_b, np.float32)
    W_att = np.asarray(W_att, np.float32); b_att = np.asarray(b_att, np.float32)
    W1 = np.asarray(W1, np.float32); b1 = np.asarray(b1, np.float32)
    W2 = np.asarray(W2, np.float32); b2 = np.asarray(b2, np.float32)
    W3 = np.asarray(W3, np.float32); b3 = np.asarray(b3, np.float32)

    n = x.shape[0]
    loops = np.arange(n, dtype=np.int64)
    src = np.concatenate([edge_index[0].astype(np.int64), loops])
    dst = np.concatenate([edge_index[1].astype(np.int64), loops])

    # sort edges by destination once; every node has a self-loop so no
    # segment is empty and reduceat boundaries are strictly increasing
    order = np.argsort(dst, kind='stable')
    src_s = src[order]
    dst_s = dst[order]
    counts = np.bincount(dst_s, minlength=n)
    starts = np.zeros(n, dtype=np.int64)
    np.cumsum(counts[:-1], out=starts[1:])

    h = np.concatenate([x, pos], axis=-1) @ W_in + b_in
    h = _silu(_bn(h, in_g, in_b))

    for i in range(L):
        hr = h
        z = (h @ Wg[i]).reshape(n, NH, HC)
        es = (z * a_src[i]).sum(-1)                      # [N,H]
        ed = (z * a_dst[i]).sum(-1)                      # [N,H]
        e = es[src_s] + ed[dst_s]
        e = np.where(e >= 0, e, np.float32(0.2) * e)     # leaky_relu
        m = np.maximum.reduceat(e, starts, axis=0)       # [N,H]
        p = np.exp(e - m[dst_s])
        denom = np.add.reduceat(p, starts, axis=0)
        alpha = p / denom[dst_s]                         # [Etot,H]
        h2 = np.empty((n, NH, HC), dtype=np.float32)
        for hh in range(NH):
            w = z[src_s, hh, :] * alpha[:, hh:hh + 1]    # [Etot,HC]
            h2[:, hh, :] = np.add.reduceat(w, starts, axis=0)
        h2 = h2.reshape(n, HID) + bg[i]
        h2 = _bn(h2, ln_g[i], ln_b[i])
        h = _silu(h2 + hr)

    # attention pooling: global softmax, then per-graph softmax of weights
    s = h @ W_att + b_att                                # [N,1]
    s = s - s.max(0)
    w = np.exp(s)
    w = w / w.sum(0)
    g = int(batch.max()) + 1 if batch.size else 0
    g = max(g, G)
    cg = np.bincount(batch, minlength=g)
    sg = np.zeros(g, dtype=np.int64)
    np.cumsum(cg[:-1], out=sg[1:])
    nonempty = cg > 0
    sg_c = np.minimum(sg, max(batch.size - 1, 0))
    mw = np.maximum.reduceat(w, sg_c, axis=0)
    mw[~nonempty] = -np.inf
    pw = np.exp(w - mw[batch])
    swsum = np.add.reduceat(pw, sg_c, axis=0)
    swsum[~nonempty] = 1.0
    sw = pw / swsum[batch]
    hw = h * sw                                          # [N,HID]
    hg = np.add.reduceat(hw, sg_c, axis=0)
    hg[~nonempty] = 0.0
    hg = hg[:G] if g > G else hg

    o = _silu(hg @ W1 + b1)
    o = _silu(o @ W2 + b2)
    pred = o @ W3 + b3
    return np.asarray(pred, np.float32), np.asarray(h, np.float32)
